# revision 1
# baseline (speedup 1.0000x reference)
"""Causal self-attention (per-head A projections) on 8 TRN2 NeuronCores.

Shapes: h [B=2, T=2048, d=64] f32, A [H=8, d, d] f32.
  q = h @ A[i]; scores = q @ h^T (causal); out_i = softmax(scores) @ h.
Sharding: one head per core (embarrassingly parallel, no collectives).
Each core receives the full h and its own A[i]; host concatenates heads.

Chunk-pipelined two-pass softmax, emitted as interleaved units so the
DVE-heavy stats pass of one chunk overlaps the ACT-heavy exp/AV pass of
earlier chunks (both batches staggered; batch 0 ascends its chunks, batch 1
descends so the post-stats tail is only the smallest chunks):

  PRE(b,c): PE-transpose h chunk into half 0 of a 2-bank PSUM tile and
      qT = A-matmul into half 1; ACT/DVE copy them out into per-batch
      merged hc/qc [65, T] f32r tiles (rounding copies - the BIR verifier
      requires f32r matmul inputs to be produced as f32r); Pool casts hs16.
  S(b,g): stats row-max for t-tiles 4g..4g+3.  All-f32r score matmuls over
      column windows of hc ([0,rem) + 512-wide windows end-aligned at the
      causal boundary); -1e30 upper-triangle accumulated via identity
      matmul on the diagonal window (start=False into the same PSUM
      region).  Window PAIRS are laid out contiguously across the two
      banks of one PSUM tile (a partial leading window packed right
      against the bank boundary) so ONE DVE reduce_max consumes each pair;
      a second tiny DVE reduce negates into -m (f32r); a [128,1] SBUF DMA
      reshapes it into row 64 of qc.
  P(b,c): pass2 scoresT tiles [s,t] (f32r, K=65 so the -m row folds the
      subtraction into the matmul), partial windows on diagonal tiles
      (>=256 wide to keep f32r single-pass) and -1e30 on the acausal
      diagonal part accumulated by an identity matmul (PE) before ACT's
      exp -> pT bf16 (keeps Pool off the exp->AV chain); AV
      accumulates NATURAL-layout out tiles [t, d+1] in PSUM with
      lhsT=pT column slices and rhs=hs16 (ones column = softmax
      denominator l) - no output transposes.  NOTE: matmul start=True
      clears has_written for the whole PSUM bank, so only the first AV
      matmul into each oT bank sets it.  Finalize: DVE reciprocal of the
      four l columns + per-tile scale (ACT/DVE), one output DMA per chunk.
"""

import os
import sys

for _p in ("/opt/trn_rl_repo",):
    if _p not in sys.path:
        sys.path.insert(0, _p)

import numpy as np
from contextlib import ExitStack

import concourse.bass as bass
import concourse.tile as tile
from concourse import bacc, mybir
from concourse.masks import make_identity
from concourse.bass_utils import run_bass_kernel_spmd

B, T, D, H = 2, 2048, 64, 8
P = 128                # square tile size (t and s)
NT = T // P            # 16 tiles along t/s
CH = 512               # chunk width (PSUM bank / fp32 moving max)
NCH = T // CH          # 4 chunks
NEG = -1e30

f32 = mybir.dt.float32
f32r = mybir.dt.float32r
bf16 = mybir.dt.bfloat16


def _stat_windows(i):
    """Column windows covering the causal region [0, 128*(i+1)) for t-tile i.

    First window is widened to >=256 columns (f32r single-pass needs >=256
    moving); remaining windows are 512 wide, end-aligned so the last one
    lands exactly on the causal boundary (where the -inf mask goes).
    Overlap between windows is harmless for a max.
    """
    s_end = (i + 1) * P
    rem = s_end % CH
    wins = []
    start = 0
    if rem:
        # <256-wide first windows pay the f32r 4x matmul penalty on the PE
        # (which has slack); DVE reduce columns stay at the causal minimum.
        wins.append((0, rem))
        start = rem
    wins.extend(
        (start + k * CH, start + (k + 1) * CH) for k in range((s_end - start) // CH)
    )
    return wins


def _build(ctx: ExitStack, tc: "tile.TileContext", h_ext, A_ext, out_ext):
    nc = tc.nc
    dbg_ot = None
    if os.environ.get("KDBG_OT"):
        dbg_ot = nc.dram_tensor(
            "dbg_ot", [B, NCH, P, 4, D + 8], f32, kind="ExternalOutput"
        ).ap()
    dbg_pt = None
    if os.environ.get("KDBG_PT"):
        dbg_pt = nc.dram_tensor(
            "dbg_pt", [4, P, CH], bf16, kind="ExternalOutput"
        ).ap()

    consts = ctx.enter_context(tc.tile_pool(name="consts", bufs=1))
    hpool = ctx.enter_context(tc.tile_pool(name="hpool", bufs=2))
    qpool = ctx.enter_context(tc.tile_pool(name="qpool", bufs=2))
    ppool = ctx.enter_context(tc.tile_pool(name="ppool", bufs=6))
    spool = ctx.enter_context(tc.tile_pool(name="spool", bufs=4))
    opool = ctx.enter_context(tc.tile_pool(name="opool", bufs=2))
    # PSUM (8 banks): stats/pre 2-bank tiles x2 + pass2 x2 + out x2.
    ps_st = ctx.enter_context(tc.tile_pool(name="ps_st", bufs=2, space="PSUM"))
    ps_p2 = ctx.enter_context(tc.tile_pool(name="ps_p2", bufs=3, space="PSUM"))
    ps_out = ctx.enter_context(tc.tile_pool(name="ps_out", bufs=1, space="PSUM"))

    # ---- constants ----
    ident = consts.tile([P, P], f32)
    make_identity(nc, ident)
    identb = consts.tile([P, P], bf16)
    make_identity(nc, identb)

    # umask[t, s] = NEG where s > t else 0 (stats-pass diagonal tile).
    umask = consts.tile([P, P], bf16)
    nc.gpsimd.memset(umask, 0.0)
    nc.gpsimd.affine_select(
        out=umask, in_=umask, compare_op=mybir.AluOpType.is_ge, fill=NEG,
        base=0, channel_multiplier=1, pattern=[[-1, P]],
    )
    # lmask[s, t] = NEG where t < s else 0 (pass-2 diagonal tile, scoresT).
    lmask = consts.tile([P, P], bf16)
    nc.gpsimd.memset(lmask, 0.0)
    nc.gpsimd.affine_select(
        out=lmask, in_=lmask, compare_op=mybir.AluOpType.is_ge, fill=NEG,
        base=0, channel_multiplier=-1, pattern=[[1, P]],
    )
    # A for this core's head: [d, e] natural layout (d on partitions), f32r.
    Asb32 = consts.tile([D, D], f32)
    nc.sync.dma_start(out=Asb32, in_=A_ext)
    Asb = consts.tile([D, D], f32r)
    nc.scalar.copy(Asb, Asb32)

    # ---- bulk input loads (both batches up front) ----
    hs32s, hs16s, hcs, qcs = [], [], [], []
    for b in range(B):
        hs32 = hpool.tile([P, NT, D + 1], f32, tag="hs32", name=f"hs32_{b}")
        h_re = h_ext[b].rearrange("(j p) d -> p j d", p=P)
        if b == 0:
            # fine-grained first-chunk loads so the first transpose starts
            # as early as possible (shorter pipeline ramp)
            for j in range(4):
                nc.sync.dma_start(out=hs32[:, j, 0:D], in_=h_re[:, j, :])
            nc.sync.dma_start(out=hs32[:, 4:NT, 0:D], in_=h_re[:, 4:NT, :])
        else:
            for c in range(NCH):
                nc.sync.dma_start(
                    out=hs32[:, 4 * c : 4 * c + 4, 0:D],
                    in_=h_re[:, 4 * c : 4 * c + 4, :],
                )
        nc.gpsimd.memset(hs32[:, :, D : D + 1], 1.0)
        hs32s.append(hs32)
        hs16s.append(hpool.tile([P, NT, D + 1], bf16, tag="hs16", name=f"hs16_{b}"))
        hcs.append(hpool.tile([D + 1, T], f32r, tag="hc", name=f"hc_{b}"))
        qcs.append(qpool.tile([D + 1, T], f32r, tag="qc", name=f"qc_{b}"))

    def emit_pre(b, c, copies_act=False):
        """Transpose h chunk + qT chunk through one 2-bank PSUM tile."""
        lo = c * CH
        cp = nc.scalar.copy if copies_act else nc.vector.tensor_copy
        cp2 = nc.scalar.copy if b == 1 else cp  # qc off DVE in the busy middle
        stile = ps_st.tile([P, 2 * CH], f32, tag="st", name=f"pre_{b}_{c}")
        pt = stile[0 : D + 1, 0:CH]
        for r in range(4):
            j = 4 * c + r
            nc.tensor.transpose(pt[:, r * P : (r + 1) * P], hs32s[b][:, j, :], ident)
        hc = hcs[b][:, lo : lo + CH]
        cp(hc, pt)

        pq = stile[0:D, CH : 2 * CH]
        nc.tensor.matmul(pq, lhsT=Asb, rhs=hc[0:D, :], start=True, stop=True)
        cp2(qcs[b][0:D, lo : lo + CH], pq)
        # hs16 s-tiles for this chunk (AV rhs): bf16 cast on Pool.
        nc.gpsimd.tensor_copy(
            hs16s[b][:, 4 * c : 4 * c + 4, :], hs32s[b][:, 4 * c : 4 * c + 4, :]
        )

    def emit_stats(b, g):
        """Row-max for t-tiles 4g..4g+3 -> -m into qc row 64."""
        for i in range(4 * g, 4 * g + 4):
            s_end = (i + 1) * P
            wins = _stat_windows(i)
            single = len(wins) <= 2  # one packed reduce -> write -m direct
            mxp = spool.tile([P, 2], f32, tag="mxp")
            negm = spool.tile([P, 1], f32r, tag="negm")
            lhs_q = qcs[b][0:D, i * P : (i + 1) * P]
            nred = 0
            for p0 in range(0, len(wins), 2):
                pair = wins[p0 : p0 + 2]
                flat = ps_st.tile([P, 2 * CH], f32, tag="st")
                # Lay the pair out so one DVE op consumes it: a partial-width
                # leading window is packed right against the bank boundary
                # (single contiguous reduce), a (512, 512) pair fills both
                # banks for a tensor_tensor_reduce.
                if len(pair) == 2:
                    (lo0, hi0), (lo1, hi1) = pair
                    w0, w1 = hi0 - lo0, hi1 - lo1
                    spans = [(lo0, hi0, CH - w0, CH), (lo1, hi1, CH, CH + w1)]
                else:
                    (lo0, hi0) = pair[0]
                    w0 = hi0 - lo0
                    spans = [(lo0, hi0, 0, w0)]
                for lo, hi, a, bnd in spans:
                    nc.tensor.matmul(
                        flat[:, a:bnd], lhsT=lhs_q, rhs=hcs[b][0:D, lo:hi],
                        start=True, stop=not hi == s_end, skip_group_check=True,
                    )
                    if hi == s_end:
                        nc.tensor.matmul(
                            flat[:, bnd - P : bnd], lhsT=identb, rhs=umask,
                            start=False, stop=True, skip_group_check=True,
                        )
                # One contiguous DVE reduce per pair (packed across the bank
                # boundary); DVE may read only ONE PSUM operand per op, so a
                # two-input tensor_tensor_reduce is not available here.
                a0 = spans[0][2]
                a1 = spans[-1][3]
                if single:
                    nc.vector.reduce_max(
                        negm, flat[:, a0:a1], axis=mybir.AxisListType.X,
                        negate=True,
                    )
                else:
                    nc.vector.reduce_max(
                        mxp[:, nred : nred + 1], flat[:, a0:a1],
                        axis=mybir.AxisListType.X,
                    )
                nred += 1
            # Second-level max + negate in one DVE op (rounded f32r output,
            # as the BIR verifier requires for f32r matmul inputs).
            if not single:
                nc.vector.reduce_max(
                    negm, mxp[:, 0:nred], axis=mybir.AxisListType.X, negate=True
                )
            # Partition-column -> free-row reshape via a tiny SBUF->SBUF DMA.
            nc.sync.dma_start(
                out=qcs[b][D : D + 1, i * P : (i + 1) * P], in_=negm
            )

    class P2Chunk:
        """Pass2 scoresT + exp + natural-layout AV + finalize for one chunk.

        Split into per-j steps so two chunks can be emitted zipper-style
        (alternating j-steps) to pipeline the ACT-bound kernel tail.
        """

        def __init__(self, b, c, scale_dve=False, scale_act=False):
            self.b, self.c, self.scale_dve = b, c, scale_dve
            self.scale_act = scale_act
            self.oT = ps_out.tile([P, 4, D + 8], f32, tag="oT", name=f"oT_{b}_{c}")
            self.osb = opool.tile([P, 4, D], f32, tag="osb", name=f"osb_{b}_{c}")
            self.av_queue = []
            self.nsteps = 4 * c + 4

        def flush_av(self, limit):
            while len(self.av_queue) > limit:
                jq, pTq = self.av_queue.pop(0)
                for k in range(4):
                    i = 4 * self.c + k
                    if jq > i:
                        continue
                    # start=True clears has_written for the WHOLE bank, so
                    # only the very first matmul into this oT bank may set
                    # it; later first-writes hit cleared bits and overwrite.
                    nc.tensor.matmul(
                        self.oT[:, k, 0 : D + 1], lhsT=pTq[:, k * P : (k + 1) * P],
                        rhs=hs16s[self.b][:, jq, :],
                        start=(jq == 0 and k == 0), stop=(jq == i),
                        skip_group_check=True,
                    )

        def step(self, j):
            b, c = self.b, self.c
            r = j - 4 * c  # >= 0 on diagonal tiles
            diag = r >= 0
            # f32r needs a >=256-wide moving window for full rate.
            wm = CH if not diag else max(CH - P * r, 2 * P)
            ws = CH - wm
            toff = 0 if not diag else P * r  # first causal t column
            p2 = ps_p2.tile([P, CH], f32, tag="p2")
            nc.tensor.matmul(
                p2[:, ws:CH],
                lhsT=hcs[b][:, j * P : (j + 1) * P],
                rhs=qcs[b][:, c * CH + ws : (c + 1) * CH],
                start=True, stop=not diag, skip_group_check=True,
            )
            if diag:
                # -1e30 on the acausal diagonal part, accumulated in PSUM by
                # the PE (keeps Pool out of the exp->AV dependency chain).
                nc.tensor.matmul(
                    p2[:, toff : toff + P], lhsT=identb, rhs=lmask,
                    start=False, stop=True, skip_group_check=True,
                )
            pT = ppool.tile([P, CH], bf16, tag="pT")
            nc.scalar.activation(
                pT[:, toff:CH], p2[:, toff:CH], mybir.ActivationFunctionType.Exp
            )
            if dbg_pt is not None and b == 0 and c == 0:
                nc.sync.dma_start(out=dbg_pt[j][:, toff:CH], in_=pT[:, toff:CH])
            self.av_queue.append((j, pT))
            self.flush_av(2)

        def finish(self):
            self.flush_av(0)
            if dbg_ot is not None:
                dts = opool.tile([P, 4, D + 1], f32, tag="dbgot")
                nc.vector.tensor_copy(dts, self.oT)
                nc.sync.dma_start(out=dbg_ot[self.b, self.c], in_=dts)
            # Finalize: strided reciprocal over the 4 l columns, then scale.
            oT, osb = self.oT, self.osb
            rl = spool.tile([P, 4], f32, tag="rl")
            for k in range(4):
                nc.vector.reciprocal(rl[:, k : k + 1], oT[:, k, D : D + 1])
            for k in range(4):
                if not self.scale_act and (self.scale_dve or k % 2 == 0):
                    nc.vector.tensor_scalar_mul(
                        osb[:, k, :], oT[:, k, 0:D], rl[:, k : k + 1]
                    )
                else:
                    nc.scalar.activation(
                        osb[:, k, :], oT[:, k, 0:D],
                        mybir.ActivationFunctionType.Copy, scale=rl[:, k : k + 1],
                    )
            nc.sync.dma_start(
                out=out_ext[self.b, self.c * CH : (self.c + 1) * CH, :].rearrange(
                    "(j p) d -> p j d", p=P
                ),
                in_=osb,
            )

    def emit_p2(b, c, scale_dve=False, scale_act=False):
        st = P2Chunk(b, c, scale_dve, scale_act)
        for j in range(st.nsteps):
            st.step(j)
        st.finish()

    def emit_p2_pair(b, c_lo, c_hi):
        """Zipper two chunks' j-loops so exp pipelines through the tail."""
        lo = P2Chunk(b, c_lo, scale_dve=True)
        hi = P2Chunk(b, c_hi, scale_dve=True)
        for j in range(hi.nsteps):
            hi.step(j)
            if j < lo.nsteps:
                lo.step(j)
            if j == lo.nsteps - 1:
                lo.finish()
        hi.finish()

    # ---- unit schedule ----
    # Batch 0 ascends (stats can start after one PRE chunk -> early DVE
    # ramp); batch 1 descends (heaviest stats/exp units mid-kernel, the
    # post-stats tail is only the two smallest chunks, zippered).
    emit_pre(0, 0)
    emit_stats(0, 0)
    emit_pre(1, 0)
    emit_pre(0, 1, copies_act=True)
    emit_stats(0, 1)
    emit_p2(0, 0, scale_act=True)
    emit_pre(0, 2, copies_act=True)
    emit_stats(0, 2)
    emit_p2(0, 1, scale_act=True)
    emit_pre(0, 3)
    emit_pre(1, 1)
    emit_stats(0, 3)
    emit_p2(0, 2)
    emit_pre(1, 2)
    emit_pre(1, 3)
    emit_stats(1, 3)
    emit_p2(0, 3)
    emit_p2(1, 3, scale_dve=True)
    emit_stats(1, 2)
    emit_p2(1, 2, scale_dve=True)
    emit_stats(1, 1)
    emit_p2(1, 1, scale_dve=True)
    emit_stats(1, 0)
    emit_p2(1, 0, scale_dve=True)


_cache = {}


def _get_nc():
    if "nc" not in _cache:
        nc = bacc.Bacc(
            "TRN2", target_bir_lowering=False, debug=False, num_devices=H
        )
        h_ext = nc.dram_tensor("h", [B, T, D], f32, kind="ExternalInput").ap()
        A_ext = nc.dram_tensor("A", [D, D], f32, kind="ExternalInput").ap()
        out_ext = nc.dram_tensor("out", [B, T, D], f32, kind="ExternalOutput").ap()
        with tile.TileContext(nc) as tc:
            with ExitStack() as ctx:
                _build(ctx, tc, h_ext, A_ext, out_ext)
        nc.compile()
        _cache["nc"] = nc
    return _cache["nc"]


def run(h, A, **kw):
    """Run on hardware; returns (full output [B,T,H*D], BassKernelResults)."""
    nc = _get_nc()
    h = np.ascontiguousarray(h, dtype=np.float32)
    A = np.ascontiguousarray(A, dtype=np.float32)
    in_maps = [{"h": h, "A": np.ascontiguousarray(A[i])} for i in range(H)]
    res = run_bass_kernel_spmd(nc, in_maps, core_ids=list(range(H)), **kw)
    out = np.concatenate([res.results[i]["out"] for i in range(H)], axis=-1)
    return out, res


def kernel(h, A):
    out, _ = run(h, A)
    return out

